# revision 8
# baseline (speedup 1.0000x reference)
"""Trainium2 Bass kernel for nn_LitePTBackbone (voxelize + scatter-min rep +
linear head + densify).

Reference semantics:
  out[i] = feat[rep(i)] @ W + coord[rep(i)] @ Wc
  rep(i) = min point id among points sharing i's voxel (floor(coord/0.02)).

Strategy (sharding_hint: spatial partition of the voxel grid):
  Host: stable-sort points by voxel key (fine spatial partition), split across
  8 cores at voxel-run boundaries, pack each core into 126 chunks of 2048
  (runs never straddle chunk boundaries; tails padded with the last point).
  Device: recompute per-axis voxel ids, same-as-prev masks, hardware segmented
  scan (tensor_tensor_scan: state = m*state + (1-m)*z) broadcasts run-start
  payloads, block-diagonal K=126 matmuls apply the [9,72] head for 7 chunks at
  a time, outputs stream to DRAM.
  Host: inverse-permute rows to original point order.
"""

import numpy as np

N = 2_000_000
C = 6
OUT = 72
NCORES = 8
L = 2048            # chunk length (scan segment)
CHUNKS = 126        # chunks per core
PCORE = L * CHUNKS  # 258048
TILES = 9
CPT = 14            # chunks per z-tile
ROWS = CPT * 9      # 126 rows per z-tile (chunk-major, 9 channels each)
FB = 128            # f-positions per output block
NFB = L // FB       # 16 output blocks per chunk-column
HALF = 7            # chunks per block-diag matmul (7*72=504 <= 512)

_cache = {}


def _build(num_devices=NCORES, repeat=1):
    import concourse.bacc as bacc
    import concourse.mybir as mybir
    import concourse.tile as tile

    f32 = mybir.dt.float32
    i32 = mybir.dt.int32
    Alu = mybir.AluOpType

    nc = bacc.Bacc("TRN2", target_bir_lowering=False, debug=False,
                   num_devices=num_devices)
    z_d = nc.dram_tensor("z", [TILES, ROWS, L], f32, kind="ExternalInput").ap()
    cxyz_d = nc.dram_tensor("cxyz", [3, CHUNKS, L], f32, kind="ExternalInput").ap()
    wbd_d = nc.dram_tensor("wbd", [2, ROWS, HALF * OUT], f32,
                           kind="ExternalInput").ap()
    rsel_d = nc.dram_tensor("rsel", [TILES, CHUNKS, ROWS], f32,
                            kind="ExternalInput").ap()
    out_d = nc.dram_tensor("out", [TILES, NFB, FB, 2 * HALF * OUT], f32,
                           kind="ExternalOutput").ap()

    with tile.TileContext(nc) as tc:
        with tc.tile_pool(name="consts", bufs=1) as cpool, \
             tc.tile_pool(name="mstage", bufs=1) as mpool, \
             tc.tile_pool(name="main", bufs=2) as pool, \
             tc.tile_pool(name="stage", bufs=4) as spool, \
             tc.tile_pool(name="psum_m", bufs=2, space="PSUM") as psum_m, \
             tc.tile_pool(name="psum_o", bufs=3, space="PSUM") as psum_o:

            wbd_t = [cpool.tile([ROWS, HALF * OUT], f32, tag=f"wbd{h}",
                                name=f"wbd{h}") for h in range(2)]
            for h in range(2):
                nc.sync.dma_start(out=wbd_t[h][:], in_=wbd_d[h])
            rsel_t = [cpool.tile([CHUNKS, ROWS], f32, tag=f"rsel{t}",
                                 name=f"rsel{t}") for t in range(TILES)]
            for t in range(TILES):
                nc.sync.dma_start(out=rsel_t[t][:], in_=rsel_d[t])
            m_all = cpool.tile([CHUNKS, L], f32)

            for rep in range(repeat):
                # ---- phase 1: m_all[c, f] = same-voxel-as-previous mask
                mt = mpool.tile([CHUNKS, L], f32, tag="mt")
                for ax in range(3):
                    # cxyz holds q = coord/0.02f (host-divided, IEEE f32).
                    # Exact floor(q): g0 = rne_cast(q); g = g0 - (g0 > q).
                    cx = mpool.tile([CHUNKS, L], f32, tag="cx")
                    nc.sync.dma_start(out=cx[:], in_=cxyz_d[ax])
                    gi = mpool.tile([CHUNKS, L], i32, tag="gi")
                    nc.vector.tensor_copy(out=gi[:], in_=cx[:])
                    gf = mpool.tile([CHUNKS, L], f32, tag="gf")
                    nc.scalar.copy(out=gf[:], in_=gi[:])
                    d = mpool.tile([CHUNKS, L], f32, tag="d")
                    nc.vector.tensor_tensor(out=d[:], in0=gf[:], in1=cx[:],
                                            op=Alu.is_gt)
                    gfl = mpool.tile([CHUNKS, L], f32, tag="gfl")
                    nc.vector.tensor_tensor(out=gfl[:], in0=gf[:], in1=d[:],
                                            op=Alu.subtract)
                    e = mpool.tile([CHUNKS, L], f32, tag="e")
                    nc.vector.memset(e[:], 0.0)
                    nc.vector.tensor_tensor(out=e[:, 1:], in0=gfl[:, 1:],
                                            in1=gfl[:, :-1], op=Alu.is_equal)
                    if ax == 0:
                        nc.vector.tensor_copy(out=mt[:], in_=e[:])
                    elif ax == 1:
                        mt2 = mpool.tile([CHUNKS, L], f32, tag="mt2")
                        nc.vector.tensor_mul(out=mt2[:], in0=mt[:], in1=e[:])
                    else:
                        nc.vector.tensor_mul(out=m_all[:], in0=mt2[:], in1=e[:])

                # ---- phase 2: per z-tile
                for t in range(TILES):
                    z_t = pool.tile([ROWS, L], f32, tag="z")
                    nc.sync.dma_start(out=z_t[:], in_=z_d[t])

                    # m9[r, f] = m_all[t*CPT + r//9, f]  (replicate via matmul)
                    m9 = pool.tile([ROWS, L], f32, tag="m9")
                    for b in range(L // 512):
                        pm = psum_m.tile([ROWS, 512], f32, tag="pm")
                        nc.tensor.matmul(
                            out=pm[:], lhsT=rsel_t[t][:],
                            rhs=m_all[:, b * 512:(b + 1) * 512],
                            start=True, stop=True)
                        nc.vector.tensor_copy(out=m9[:, b * 512:(b + 1) * 512],
                                              in_=pm[:])

                    notm9 = pool.tile([ROWS, L], f32, tag="notm9")
                    nc.vector.tensor_scalar(out=notm9[:], in0=m9[:],
                                            scalar1=-1.0, scalar2=1.0,
                                            op0=Alu.mult, op1=Alu.add)
                    zm = pool.tile([ROWS, L], f32, tag="zm")
                    nc.vector.tensor_mul(out=zm[:], in0=z_t[:], in1=notm9[:])
                    zs = pool.tile([ROWS, L], f32, tag="zs")
                    nc.vector.tensor_tensor_scan(out=zs[:], data0=m9[:],
                                                 data1=zm[:], initial=0.0,
                                                 op0=Alu.mult, op1=Alu.add)

                    # out[p, ci*72+k] for 14 chunks x 128 f-positions per block
                    for b in range(NFB):
                        po = psum_o.tile([FB, 1024], f32, tag="po")
                        for h in range(2):
                            nc.tensor.matmul(
                                out=po[:, h * 512:h * 512 + HALF * OUT],
                                lhsT=zs[:, b * FB:(b + 1) * FB],
                                rhs=wbd_t[h][:], start=True, stop=True)
                        st = spool.tile([FB, 2 * HALF * OUT], f32, tag="st")
                        eng_v = (b % 2 == 0)
                        cp = nc.vector.tensor_copy if eng_v else nc.scalar.copy
                        cp(out=st[:, 0:504], in_=po[:, 0:504])
                        cp(out=st[:, 504:1008], in_=po[:, 512:1016])
                        nc.sync.dma_start(out=out_d[t, b], in_=st[:])
    nc.compile()
    return nc


def _get_nc(repeat=1):
    key = ("nc", repeat)
    if key not in _cache:
        _cache[key] = _build(NCORES, repeat)
    return _cache[key]


def _host_shard(coord, feat):
    """Sort by voxel key, split across cores at run boundaries, pack chunks."""
    coord = np.ascontiguousarray(coord, np.float32)
    feat = np.ascontiguousarray(feat, np.float32)
    n = coord.shape[0]
    # voxel ids exactly as reference and device: floor(x / 0.02f) in f32
    g = np.floor(coord / np.float32(0.02)).astype(np.int64)
    key = (g[:, 0] << 42) | (g[:, 1] << 21) | g[:, 2]
    order = np.argsort(key, kind="stable")
    ks = key[order]
    newrun = np.empty(n, bool)
    newrun[0] = True
    np.not_equal(ks[1:], ks[:-1], out=newrun[1:])
    run_starts = np.flatnonzero(newrun)

    bounds = [0]
    for k in range(1, NCORES):
        tgt = k * n // NCORES
        rb = run_starts[np.searchsorted(run_starts, tgt, side="right") - 1]
        bounds.append(int(rb))
    bounds.append(n)

    IDX = np.empty((NCORES, CHUNKS, L), np.int64)
    for k in range(NCORES):
        s0, s1 = bounds[k], bounds[k + 1]
        assert s1 - s0 <= PCORE, f"shard {k} too big: {s1 - s0}"
        pos = s0
        for c in range(CHUNKS):
            if pos >= s1:
                IDX[k, c, :] = order[s1 - 1]
                continue
            lim = pos + L
            if lim >= s1:
                end = s1
            else:
                jj = np.searchsorted(run_starts, lim, side="right") - 1
                end = int(run_starts[jj])
                assert end > pos, "voxel run longer than chunk"
            fill = end - pos
            IDX[k, c, :fill] = order[pos:end]
            IDX[k, c, fill:] = order[end - 1]
            pos = end
        assert pos == s1, (k, pos, s1)
    return IDX, coord, feat


def _prep_in_maps(coord, feat, W, Wc):
    IDX, coord32, feat32 = _host_shard(coord, feat)
    payload = np.concatenate([feat32, coord32], axis=1)  # [N, 9]
    wfull = np.concatenate(
        [np.ascontiguousarray(W, np.float32),
         np.ascontiguousarray(Wc, np.float32)], axis=0)  # [9, 72]

    wbd = np.zeros((2, ROWS, HALF * OUT), np.float32)
    for ci in range(CPT):
        h, cl = divmod(ci, HALF)
        wbd[h, ci * 9:(ci + 1) * 9, cl * OUT:(cl + 1) * OUT] = wfull
    rsel = np.zeros((TILES, CHUNKS, ROWS), np.float32)
    for t in range(TILES):
        for r in range(ROWS):
            rsel[t, t * CPT + r // 9, r] = 1.0

    in_maps = []
    for k in range(NCORES):
        zp = payload[IDX[k]]                             # [CHUNKS, L, 9]
        Z = np.ascontiguousarray(
            zp.reshape(TILES, CPT, L, 9).transpose(0, 1, 3, 2)
        ).reshape(TILES, ROWS, L)
        CX = np.ascontiguousarray(
            (zp[:, :, 6:9] / np.float32(0.02)).transpose(2, 0, 1))
        in_maps.append({"z": Z, "cxyz": CX, "wbd": wbd, "rsel": rsel})
    return IDX, in_maps


def _decode_out(res_core):
    # out [TILES, NFB, FB, 1008] -> rows in chunk-major point order
    arr = res_core.reshape(TILES, NFB, FB, CPT, OUT)
    return np.ascontiguousarray(arr.transpose(0, 3, 1, 2, 4)).reshape(PCORE, OUT)


def kernel(coord, feat, W, Wc):
    coord_in = np.asarray(coord)
    feat_in = np.asarray(feat)
    n = coord_in.shape[0]
    if n != N or feat_in.shape[1] != C:
        return _host_fallback(coord_in, feat_in,
                              np.asarray(W, np.float32),
                              np.asarray(Wc, np.float32))

    from concourse import bass_utils

    IDX, in_maps = _prep_in_maps(coord_in, feat_in, W, Wc)
    nc = _get_nc()
    res = bass_utils.run_bass_kernel_spmd(nc, in_maps, list(range(NCORES)))

    out_full = np.empty((n, OUT), np.float32)
    for k in range(NCORES):
        out_full[IDX[k].reshape(-1)] = _decode_out(res.results[k]["out"])
    return out_full


def _host_fallback(coord, feat, W, Wc):
    """Pure-numpy replica of the reference for unexpected shapes."""
    coord = coord.astype(np.float32)
    feat = feat.astype(np.float32)
    grid = np.floor(coord / np.float32(0.02)).astype(np.int32)
    grid = grid - grid.min(axis=0)
    gmax = grid.max(axis=0) + 1
    keys = (grid[:, 0].astype(np.int64) * gmax[1] + grid[:, 1]) * gmax[2] + grid[:, 2]
    _, inv = np.unique(keys, return_inverse=True)
    first = np.full(inv.max() + 1, 1 << 60, np.int64)
    np.minimum.at(first, inv, np.arange(coord.shape[0]))
    rep = first[inv]
    return feat[rep] @ W + coord[rep] @ Wc
